# revision 1
# baseline (speedup 1.0000x reference)
"""Causal bilinear self-attention kernel for 8 Trainium2 NeuronCores.

Sharding: core c handles batch b = c//4 and head group g = c%4 (4 of 16
heads, feature slice [256g, 256g+256)).  Each core computes its partial
output-projection contribution y_partial = z_slice @ Wproj[:, slice].T and
the host sums the 4 partials per batch.

All matmuls run in float32r (TF32-like, full PE rate at N>=256).
RoPE is folded into extra sign-permuted-weight projections so it becomes
3 full-width DVE ops per tensor chunk:  q_roped = q*C + q_sw*S.

Schedule: phase A (projections) is split in t-halves; the second half is
interleaved into the first two q-block columns of phase B (scores), which
only depend on the first half.  Scores/z matmuls pack both heads of a
chunk into concurrent row/col tile_position groups.  Output projection is
interleaved per q-block as soon as all heads' z for that block are done.
"""

import numpy as np

import concourse.tile as tile
from concourse import bacc, mybir
from concourse.bass_utils import run_bass_kernel_spmd

D_MODEL = 1024
N_HEAD = 16
HEAD_DIM = 64  # Dh
B, T = 2, 2048
ROPE_BASE = 10000.0
N_CORES = 8
HG = 4          # heads per core
FS = HG * HEAD_DIM  # 256 features per core

F32 = mybir.dt.float32
F32R = mybir.dt.float32r

_PROGRAM = None


def _build_program():
    nc = bacc.Bacc("TRN2", target_bir_lowering=False, debug=False)

    xt_d = nc.dram_tensor("xt", [D_MODEL, T], F32, kind="ExternalInput").ap()
    wpk_d = nc.dram_tensor("wpk", [9, 128, 2048], F32, kind="ExternalInput").ap()
    wpj_d = nc.dram_tensor("wpj", [128, 2048], F32, kind="ExternalInput").ap()
    tabs_d = nc.dram_tensor("tabs", [3, 128, 2048], F32, kind="ExternalInput").ap()
    y_d = nc.dram_tensor("y", [T, D_MODEL], F32, kind="ExternalOutput").ap()

    with tile.TileContext(nc) as tc:
        with (
            tc.tile_pool(name="pers", bufs=1) as pers,
            tc.tile_pool(name="xp", bufs=8) as xp,
            tc.tile_pool(name="wp", bufs=2) as wp,
            tc.tile_pool(name="swp", bufs=2) as swp,
            tc.tile_pool(name="mkp", bufs=1) as mkp,
            tc.tile_pool(name="ppl", bufs=3) as ppl,
            tc.tile_pool(name="sbp", bufs=3) as sbp,
            tc.tile_pool(name="ysb", bufs=4) as ysb,
            tc.tile_pool(name="psA", bufs=2, space="PSUM") as psA,
            tc.tile_pool(name="psS", bufs=2, space="PSUM") as psS,
            tc.tile_pool(name="psS2", bufs=2, space="PSUM") as psS2,
            tc.tile_pool(name="psZ", bufs=2, space="PSUM") as psZ,
        ):
            # persistent tiles
            proj = [pers.tile([128, T], F32R, tag=f"proj{i}", name=f"proj{i}")
                    for i in range(8)]
            # proj[2*ti+f] = chunk f of tensor ti (0=q,1=k,2=q2,3=k2)
            vt = [pers.tile([128, 1024], F32R, tag=f"v{i}", name=f"v{i}")
                  for i in range(4)]
            zt = [pers.tile([128, T], F32R, tag=f"z{i}", name=f"z{i}")
                  for i in range(2)]
            ctab = pers.tile([128, 2048], F32R, tag="ctab")
            stab = pers.tile([128, 2048], F32R, tag="stab")
            masks = mkp.tile([128, 2048], F32, tag="masks")
            wpjt = pers.tile([128, 2048], F32R, tag="wpjt")

            def emit_A(th, f_sel=(0, 1), v_sel=range(8)):
                """Generator: projection units for t-half `th`.

                Yields after each PSUM-tile unit so phase B blocks can be
                interleaved between units."""
                # prefetch the first weight so PE can start after ~1.5 MB of DMA
                wt0 = wp.tile([128, 2048], F32R, tag="wt", name="wt")
                nc.sync.dma_start(wt0[:], wpk_d[0].bitcast(F32R))
                xtiles = []
                for kc in range(8):
                    xtl = xp.tile([128, 1024], F32R, tag="xt", name="xtl")
                    nc.sync.dma_start(
                        xtl[:],
                        xt_d[kc * 128 : kc * 128 + 128,
                             th * 1024 : th * 1024 + 1024].bitcast(F32R),
                    )
                    xtiles.append(xtl)

                def proj_unit(wt, f, tt, dst_ap):
                    ps = psA.tile([128, 512], F32, tag="psa", name="psa")
                    for kc in range(8):
                        nc.tensor.matmul(
                            ps[:],
                            wt[:, kc * 256 + f * 128 : kc * 256 + f * 128 + 128],
                            xtiles[kc][:, tt * 512 : tt * 512 + 512],
                            start=(kc == 0),
                            stop=(kc == 7),
                        )
                    nc.scalar.copy(dst_ap, ps[:])

                for wi in range(8):  # q, qsw, k, ksw, q2, q2sw, k2, k2sw
                    if wi == 0:
                        wt = wt0
                    else:
                        wt = wp.tile([128, 2048], F32R, tag="wt", name="wt")
                        nc.sync.dma_start(wt[:], wpk_d[wi].bitcast(F32R))
                    if th == 0 and wi == 1:
                        # rope tables needed from the first sw unit on
                        nc.sync.dma_start(ctab[:], tabs_d[0].bitcast(F32R))
                        nc.sync.dma_start(stab[:], tabs_d[1].bitcast(F32R))
                    if th == 0 and wi == 5:
                        # mask + output-projection tables only needed in B/C
                        nc.sync.dma_start(masks[:], tabs_d[2])
                        nc.sync.dma_start(wpjt[:], wpj_d[:].bitcast(F32R))
                    ti, is_sw = wi // 2, wi % 2 == 1
                    for f in f_sel:
                        if is_sw:
                            dst = swp.tile([128, 1024], F32R, tag="swt", name="swt")
                            for tt in range(2):
                                proj_unit(wt, f, tt, dst[:, tt * 512 : tt * 512 + 512])
                                yield
                            # rope for tensor ti, chunk f, this t-half
                            psl = proj[2 * ti + f][:, th * 1024 : th * 1024 + 1024]
                            cs = ctab[:, th * 1024 : th * 1024 + 1024]
                            ss = stab[:, th * 1024 : th * 1024 + 1024]
                            nc.vector.tensor_mul(dst[:], dst[:], ss)
                            nc.vector.tensor_mul(psl, psl, cs)
                            nc.vector.tensor_add(psl, psl, dst[:])
                            yield
                        else:
                            for tt in range(2):
                                col = th * 1024 + tt * 512
                                proj_unit(
                                    wt, f, tt,
                                    proj[2 * ti + f][:, col : col + 512],
                                )
                                yield

                # v projection: natural layout [t, f]
                wt = wp.tile([128, 2048], F32R, tag="wt", name="wt")
                nc.sync.dma_start(wt[:], wpk_d[8].bitcast(F32R))
                for m in v_sel:
                    tg = th * 8 + m  # global 128-row t tile
                    psv = psA.tile([128, 512], F32, tag="psa", name="psa")
                    for kc in range(8):
                        nc.tensor.matmul(
                            psv[:, 0:256],
                            xtiles[kc][:, m * 128 : m * 128 + 128],
                            wt[:, kc * 256 : kc * 256 + 256],
                            start=(kc == 0),
                            stop=(kc == 7),
                        )
                    nc.scalar.copy(
                        vt[tg // 4][:, (tg % 4) * 256 : (tg % 4) * 256 + 256],
                        psv[:, 0:256],
                    )
                    yield

            def drain(gen, n):
                for _ in range(n):
                    try:
                        next(gen)
                    except StopIteration:
                        return False
                return True

            def emit_B_block(qq, hp):
                """Scores + bilinear + z for q-block qq, head pair hp.

                Heads 2hp (rows 0:64 of chunk hp) and 2hp+1 (rows 64:128)
                are packed into concurrent row/col tile_position groups.
                Diagonal tiles are narrowed to their unmasked column range."""
                ch = hp
                kT, qT = proj[2 + ch], proj[0 + ch]
                k2T, q2T = proj[6 + ch], proj[4 + ch]
                zps = [
                    psZ.tile([64, 512], F32, tag="zps", name="zps")
                    for _ in range(2)
                ]
                last = 4 * qq + 3

                def scores(kk):
                    off = max(0, kk - 4 * qq) * 128  # first unmasked col
                    qsl = slice(qq * 512 + off, qq * 512 + 512)
                    ksl = slice(kk * 128, kk * 128 + 128)
                    sps = [None, None]
                    s2ps = [None, None]
                    for hh in range(2):
                        rb = 64 * hh
                        sp = psS.tile([128, 512], F32, tag="sps", name="sps")
                        nc.tensor.matmul(
                            sp[:, off:512],
                            kT[rb : rb + 64, ksl],
                            qT[rb : rb + 64, qsl],
                            start=True, stop=True,
                            tile_position=(rb, 0),
                        )
                        sps[hh] = sp
                    for hh in range(2):
                        rb = 64 * hh
                        s2 = psS2.tile([128, 512], F32, tag="s2ps", name="s2ps")
                        nc.tensor.matmul(
                            s2[:, off:512],
                            k2T[rb : rb + 64, ksl],
                            q2T[rb : rb + 64, qsl],
                            start=True, stop=True,
                            tile_position=(rb, 0),
                        )
                        s2ps[hh] = s2
                    return sps, s2ps

                def bilinear_z(kk, sps, s2ps):
                    off = max(0, kk - 4 * qq) * 128
                    for hh in range(2):
                        pt = ppl.tile([128, 512], F32R, tag="pt", name="pt")
                        tmp = sbp.tile([128, 512], F32, tag="tmp", name="tmp")
                        if kk >= 4 * qq:
                            moff = kk - 4 * qq
                            # true-diagonal 128-col strip: masked, 2 DVE ops
                            nc.vector.tensor_mul(
                                tmp[:, off : off + 128],
                                sps[hh][:, off : off + 128],
                                masks[:, moff * 512 + off : moff * 512 + off + 128],
                            )
                            nc.vector.tensor_mul(
                                pt[:, off : off + 128],
                                tmp[:, off : off + 128],
                                s2ps[hh][:, off : off + 128],
                            )
                            if off + 128 < 512:
                                # below-diagonal remainder: unmasked product
                                nc.scalar.copy(
                                    tmp[:, off + 128 : 512],
                                    sps[hh][:, off + 128 : 512],
                                )
                                nc.vector.tensor_mul(
                                    pt[:, off + 128 : 512],
                                    tmp[:, off + 128 : 512],
                                    s2ps[hh][:, off + 128 : 512],
                                )
                        else:
                            nc.scalar.copy(tmp[:, off:512], sps[hh][:, off:512])
                            nc.vector.tensor_mul(
                                pt[:, off:512], tmp[:, off:512],
                                s2ps[hh][:, off:512],
                            )
                        nc.tensor.matmul(
                            zps[hh][:, off:512],
                            vt[kk // 4][:, (kk % 4) * 256 + (2 * hp + hh) * 64
                                        : (kk % 4) * 256 + (2 * hp + hh) * 64 + 64],
                            pt[:, off:512],
                            start=(kk == 0),
                            stop=(kk == last),
                        )

                # software pipeline: scores one kk ahead of bilinear/z
                prev = scores(0)
                for kk in range(1, last + 1):
                    cur = scores(kk)
                    bilinear_z(kk - 1, *prev)
                    prev = cur
                bilinear_z(last, *prev)
                for hh in range(2):
                    nc.scalar.copy(
                        zt[ch][64 * hh : 64 * hh + 64, qq * 512 : qq * 512 + 512],
                        zps[hh][:],
                    )

            def emit_B_gen(qq, hp, zpool, ztag):
                """Non-pipelined B block as a per-kk generator, for zipping
                two independent blocks so each fills the other's stalls.
                z accumulators come from `zpool` (psA is free in B23)."""
                ch = hp
                kT, qT = proj[2 + ch], proj[0 + ch]
                k2T, q2T = proj[6 + ch], proj[4 + ch]
                zps = [zpool.tile([64, 512], F32, tag=ztag, name="zpg")
                       for _ in range(2)]
                last = 4 * qq + 3
                for kk in range(last + 1):
                    off = max(0, kk - 4 * qq) * 128
                    qsl = slice(qq * 512 + off, qq * 512 + 512)
                    ksl = slice(kk * 128, kk * 128 + 128)
                    sps = [None, None]
                    s2ps = [None, None]
                    for hh in range(2):
                        rb = 64 * hh
                        sp = psS.tile([128, 512], F32, tag="sps", name="sps")
                        nc.tensor.matmul(
                            sp[:, off:512], kT[rb : rb + 64, ksl],
                            qT[rb : rb + 64, qsl],
                            start=True, stop=True, tile_position=(rb, 0),
                        )
                        sps[hh] = sp
                    for hh in range(2):
                        rb = 64 * hh
                        s2 = psS2.tile([128, 512], F32, tag="s2ps", name="s2ps")
                        nc.tensor.matmul(
                            s2[:, off:512], k2T[rb : rb + 64, ksl],
                            q2T[rb : rb + 64, qsl],
                            start=True, stop=True, tile_position=(rb, 0),
                        )
                        s2ps[hh] = s2
                    for hh in range(2):
                        pt = ppl.tile([128, 512], F32R, tag="pt", name="pt")
                        tmp = sbp.tile([128, 512], F32, tag="tmp", name="tmp")
                        if kk >= 4 * qq:
                            moff = kk - 4 * qq
                            nc.vector.tensor_mul(
                                tmp[:, off : off + 128],
                                sps[hh][:, off : off + 128],
                                masks[:, moff * 512 + off : moff * 512 + off + 128],
                            )
                            nc.vector.tensor_mul(
                                pt[:, off : off + 128],
                                tmp[:, off : off + 128],
                                s2ps[hh][:, off : off + 128],
                            )
                            if off + 128 < 512:
                                nc.scalar.copy(
                                    tmp[:, off + 128 : 512],
                                    sps[hh][:, off + 128 : 512],
                                )
                                nc.vector.tensor_mul(
                                    pt[:, off + 128 : 512],
                                    tmp[:, off + 128 : 512],
                                    s2ps[hh][:, off + 128 : 512],
                                )
                        else:
                            nc.scalar.copy(tmp[:, off:512], sps[hh][:, off:512])
                            nc.vector.tensor_mul(
                                pt[:, off:512], tmp[:, off:512],
                                s2ps[hh][:, off:512],
                            )
                        nc.tensor.matmul(
                            zps[hh][:, off:512],
                            vt[kk // 4][:, (kk % 4) * 256 + (2 * hp + hh) * 64
                                        : (kk % 4) * 256 + (2 * hp + hh) * 64 + 64],
                            pt[:, off:512],
                            start=(kk == 0), stop=(kk == last),
                        )
                    yield
                for hh in range(2):
                    nc.scalar.copy(
                        zt[ch][64 * hh : 64 * hh + 64, qq * 512 : qq * 512 + 512],
                        zps[hh][:],
                    )

            def emit_C(qq, ypool, ytag):
                """Output projection for the 4 t-tiles of q-block qq."""
                for tg in range(4 * qq, 4 * qq + 4):
                    for oo in range(2):
                        yps = ypool.tile([128, 512], F32, tag=ytag, name="yps")
                        for ci in range(2):
                            nc.tensor.matmul(
                                yps[:],
                                zt[ci][:, tg * 128 : tg * 128 + 128],
                                wpjt[:, ci * 1024 + oo * 512
                                     : ci * 1024 + oo * 512 + 512],
                                start=(ci == 0),
                                stop=(ci == 1),
                            )
                        yo = ysb.tile([128, 512], F32, tag="yo", name="yo")
                        nc.scalar.copy(yo[:], yps[:])
                        nc.sync.dma_start(
                            y_d[tg * 128 : tg * 128 + 128,
                                oo * 512 : oo * 512 + 512],
                            yo[:],
                        )

            # ---------------- emission schedule ----------------
            for _ in emit_A(0):
                pass
            gen1 = emit_A(1)
            for qq in range(2):
                for hp in range(2):
                    emit_B_block(qq, hp)
                    drain(gen1, 11)
                emit_C(qq, psA, "psa")
            emit_B_block(2, 0)
            drain(gen1, 1000)  # flush remaining v units (qq3-only deps)
            # interleave the two independent blocks B(2,1) and B(3,0):
            # different proj chunks -> two independent chains; B(3,0)'s z
            # accumulators use the otherwise-idle psA banks
            ga = emit_B_gen(2, 1, psZ, "zps")
            gb = emit_B_gen(3, 0, psA, "psa")
            alive = [ga, gb]
            while alive:
                for g in list(alive):
                    try:
                        next(g)
                    except StopIteration:
                        alive.remove(g)
            emit_C(2, psZ, "zps")
            emit_B_block(3, 1)
            emit_C(3, psA, "psa")

    nc.compile()
    return nc


def _get_program():
    global _PROGRAM
    if _PROGRAM is None:
        _PROGRAM = _build_program()
    return _PROGRAM


def _pack_w(ws):
    """ws: [256, 1024] (out-feature rows, in-feature cols) ->
    packed [128, 2048] where chunk kc, half f lives at cols
    [kc*256 + f*128, ...+128): lhsT tile = ws.T[kc*128:(kc+1)*128, f*128:...]."""
    a = np.ascontiguousarray(ws.T)  # [1024, 256]
    return np.ascontiguousarray(
        a.reshape(8, 128, 256).transpose(1, 0, 2).reshape(128, 2048)
    )


def _sw_w(ws):
    """Sign-permuted weight so that (x @ sw(W).T)[t, f] =
    +q[t, f+32] for d<32 and -q[t, f-32] for d>=32 (per 64-head-block)."""
    v = ws.reshape(HG, 2, 32, D_MODEL)
    out = np.empty_like(v)
    out[:, 0] = v[:, 1]
    out[:, 1] = -v[:, 0]
    return out.reshape(FS, D_MODEL)


def _make_tabs():
    inv = 1.0 / (ROPE_BASE ** (np.arange(0, HEAD_DIM, 2, dtype=np.float32) / HEAD_DIM))
    t = np.arange(T, dtype=np.float32)
    ang = np.outer(t, inv)  # [T, 32]
    c32 = np.cos(ang).astype(np.float32).T  # [32, T]
    s32 = np.sin(ang).astype(np.float32).T
    ctab = np.tile(c32, (4, 1))  # [128, T]
    stab = np.tile(s32, (4, 1))
    r = np.arange(128)[:, None]
    ccol = np.arange(512)[None, :]
    masks = np.concatenate(
        [(ccol >= r + 128 * j).astype(np.float32) for j in range(4)], axis=1
    )  # [128, 2048]
    return np.ascontiguousarray(np.stack([ctab, stab, masks]))


def kernel(x, Wq, Wk, Wq2, Wk2, Wv, Wproj):
    x = np.asarray(x, dtype=np.float32)
    Wq = np.asarray(Wq, dtype=np.float32)
    Wk = np.asarray(Wk, dtype=np.float32)
    Wq2 = np.asarray(Wq2, dtype=np.float32)
    Wk2 = np.asarray(Wk2, dtype=np.float32)
    Wv = np.asarray(Wv, dtype=np.float32)
    Wproj = np.asarray(Wproj, dtype=np.float32)

    nc = _get_program()
    tabs = _make_tabs()

    in_maps = []
    for c in range(N_CORES):
        b, g = divmod(c, HG)
        fsl = slice(g * FS, g * FS + FS)
        wq_s = Wq[fsl] * (1.0 / HEAD_DIM)
        wk_s = Wk[fsl]
        wq2_s = Wq2[fsl] * (1.0 / HEAD_DIM)
        wk2_s = Wk2[fsl]
        wv_s = Wv[fsl]
        wpk = np.ascontiguousarray(
            np.stack(
                [
                    _pack_w(wq_s), _pack_w(_sw_w(wq_s)),
                    _pack_w(wk_s), _pack_w(_sw_w(wk_s)),
                    _pack_w(wq2_s), _pack_w(_sw_w(wq2_s)),
                    _pack_w(wk2_s), _pack_w(_sw_w(wk2_s)),
                    _pack_w(wv_s),
                ]
            )
        )
        wpj = np.ascontiguousarray(
            Wproj[:, fsl].T.reshape(2, 128, 1024).transpose(1, 0, 2).reshape(128, 2048)
        )
        xt = np.ascontiguousarray(x[b].T)
        in_maps.append({"xt": xt, "wpk": wpk, "wpj": wpj, "tabs": tabs})

    res = run_bass_kernel_spmd(nc, in_maps, list(range(N_CORES))).results

    y = np.zeros((B, T, D_MODEL), dtype=np.float64)
    for c in range(N_CORES):
        b = c // HG
        y[b] += res[c]["y"].astype(np.float64)
    return y.astype(np.float32)

